# revision 4
# baseline (speedup 1.0000x reference)
"""Distributed Bass kernel for nn_Attention_94489280516 on 8 TRN2 NeuronCores.

Reference computation:
    q = x@Wq.T+bq; k = x@Wk.T+bk; v = x@Wv.T+bv          (x: [8192, 256])
    attn = softmax_global((q @ k.T) / 8192)               ([8192, 8192])
    out  = attn @ v                                       ([8192, 256])

Distribution: rows of q (and output) are sharded 1024/core across 8 cores;
K and V are computed replicated on every core from a replicated x^T. The
global softmax normalizer needs one tiny AllReduce of sum(exp(a)).

Math used on-device (no max subtraction needed: |a| < 0.03 structurally,
since a = (q.k)/8192 with q,k ~ N(0,1)-scaled):
    exp(a) = 1 + g,  g = exp(a) - 1   (computed f32, stored bf16: keeps the
                                       tiny attention signal at full relative
                                       precision through the bf16 AV matmul)
    out_rows = (colsum(V) + G @ V) / sum_global(exp(a))
colsum(V) is computed via an exact f32 path (colsum of x in f32, then a tiny
f32 matmul with Wv^T), because the output is dominated by this term.

Everything runs in one NEFF per core: projections (bf16 matmuls), flash-style
attention loop over 64 key-tiles (S^T tile -> exp -> AV accumulate, software
pipelined so the PE never stalls), epilogue with the AllReduce + rescale.
"""

import os
import sys

for _p in ("/opt/trn_rl_repo", "/root/.axon_site/_ro/trn_rl_repo"):
    if os.path.isdir(_p) and _p not in sys.path:
        sys.path.insert(0, _p)

import numpy as np

import concourse.bass as bass
import concourse.bacc as bacc
import concourse.mybir as mybir
import concourse.tile as tile
from concourse.bass_utils import run_bass_kernel_spmd

F32 = mybir.dt.float32
BF16 = mybir.dt.bfloat16
AF = mybir.ActivationFunctionType
ALU = mybir.AluOpType
AX = mybir.AxisListType

L = 8192          # total rows
C = 256           # channels
NCORES = 8
R = L // NCORES   # 1024 rows per core
P = 128           # partitions
NCH = 4           # x^T processed in NCH chunks of CHW columns
CHW = L // NCH    # 2048
JT = L // P       # 64 key tiles
JPC = JT // NCH   # 16 key tiles per chunk


def build():
    nc = bacc.Bacc(None, num_devices=NCORES)

    xT_d = nc.declare_dram_parameter("xT", [C, L], F32, isOutput=False)
    xo_d = nc.declare_dram_parameter("xTown", [C, R], F32, isOutput=False)
    wq_d = nc.declare_dram_parameter("WqT", [C, C], F32, isOutput=False)
    wk_d = nc.declare_dram_parameter("WkT", [C, C], F32, isOutput=False)
    wv_d = nc.declare_dram_parameter("WvT", [C, C], F32, isOutput=False)
    bq_d = nc.declare_dram_parameter("bq2", [C, 1], F32, isOutput=False)
    bk_d = nc.declare_dram_parameter("bk2", [C, 1], F32, isOutput=False)
    bvL_d = nc.declare_dram_parameter("bvL", [C, 1], F32, isOutput=False)
    bvr_d = nc.declare_dram_parameter("bvrep", [P, C], F32, isOutput=False)
    out_d = nc.declare_dram_parameter("out", [C, R], F32, isOutput=True)

    with tile.TileContext(nc) as tc:
        with (
            tc.tile_pool(name="const", bufs=1) as const,
            tc.tile_pool(name="big", bufs=1) as big,
            tc.tile_pool(name="dram", bufs=1, space="DRAM") as dram,
        ):
            # ---- constants / persistent tiles ----
            wq_f = const.tile([P, 2, C], F32)
            wk_f = const.tile([P, 2, C], F32)
            wv_f = const.tile([P, 2, C], F32)
            wq_b = const.tile([P, 2, C], BF16)
            wk_b = const.tile([P, 2, C], BF16)
            wv_b = const.tile([P, 2, C], BF16)
            bq_sb = const.tile([P, 2, 1], F32)
            bk_sb = const.tile([P, 2, 1], F32)
            bvL_sb = const.tile([P, 2, 1], F32)
            bvr_sb = const.tile([P, C], F32)
            ones_col = const.tile([P, 1], F32)
            ones_row = const.tile([1, P], F32)
            serow = const.tile([P, JT], F32)
            xcs = const.tile([P, 2, NCH], F32)
            colx = const.tile([P, 2, 1], F32)
            cv_sb = const.tile([P, 2, 1], F32)
            stats = const.tile([1, 8], F32)
            sg = const.tile([1, 8], F32)
            inv1 = const.tile([1, 1], F32)
            invb = const.tile([P, 1], F32)
            qT_sb = big.tile([P, 2, R], BF16)
            kT_sb = [big.tile([P, 2, CHW], BF16, name=f"kT{i}") for i in range(NCH)]
            v_sb = [big.tile([P, JPC, C], BF16, name=f"v{i}") for i in range(NCH)]
            out_sb = const.tile([P, 2, R], F32)

            for kc in range(2):
                nc.sync.dma_start(wq_f[:, kc, :], wq_d[kc * P:(kc + 1) * P, :])
                nc.sync.dma_start(wk_f[:, kc, :], wk_d[kc * P:(kc + 1) * P, :])
                nc.sync.dma_start(wv_f[:, kc, :], wv_d[kc * P:(kc + 1) * P, :])
                nc.sync.dma_start(bq_sb[:, kc, :], bq_d[kc * P:(kc + 1) * P, :])
                nc.sync.dma_start(bk_sb[:, kc, :], bk_d[kc * P:(kc + 1) * P, :])
                nc.sync.dma_start(bvL_sb[:, kc, :], bvL_d[kc * P:(kc + 1) * P, :])
            nc.sync.dma_start(bvr_sb[:], bvr_d[:, :])
            nc.vector.tensor_copy(wq_b[:], wq_f[:])
            nc.vector.tensor_copy(wk_b[:], wk_f[:])
            nc.vector.tensor_copy(wv_b[:], wv_f[:])
            nc.vector.memset(ones_col[:], 1.0)
            nc.vector.memset(ones_row[:], 1.0)
            nc.vector.memset(stats[:], 0.0)

            # ---- phase A: projections ----
            with (
                tc.tile_pool(name="xload", bufs=2) as xload,
                tc.tile_pool(name="psA", bufs=4, space="PSUM") as psA,
                tc.tile_pool(name="psA2", bufs=4, space="PSUM") as psA2,
            ):
                # q projection from this core's own row block
                xo_f = xload.tile([P, 2, R], F32, tag="xof", bufs=1)
                xo_b = xload.tile([P, 2, R], BF16, tag="xob", bufs=1)
                for kc in range(2):
                    nc.sync.dma_start(xo_f[:, kc, :], xo_d[kc * P:(kc + 1) * P, :])
                nc.vector.tensor_copy(xo_b[:], xo_f[:])
                for mc in range(2):
                    for rn in range(R // 512):
                        qps = psA.tile([P, 512], F32, tag="ps512")
                        for kc in range(2):
                            nc.tensor.matmul(
                                qps[:],
                                wq_b[:, kc, mc * P:(mc + 1) * P],
                                xo_b[:, kc, rn * 512:(rn + 1) * 512],
                                start=(kc == 0),
                                stop=(kc == 1),
                            )
                        nc.scalar.activation(
                            qT_sb[:, mc, rn * 512:(rn + 1) * 512], qps[:],
                            AF.Identity, bias=bq_sb[:, mc, :],
                        )

                # k^T and V from replicated x^T, chunk by chunk
                for ch in range(NCH):
                    xf = xload.tile([P, 2, CHW], F32, tag="xf")
                    for kc in range(2):
                        nc.sync.dma_start(
                            xf[:, kc, :],
                            xT_d[kc * P:(kc + 1) * P, ch * CHW:(ch + 1) * CHW],
                        )
                    xb = xload.tile([P, 2, CHW], BF16, tag="xb")
                    nc.vector.tensor_copy(xb[:], xf[:])
                    for kc in range(2):
                        nc.vector.tensor_reduce(
                            xcs[:, kc, ch:ch + 1], xf[:, kc, :], AX.X, ALU.add
                        )
                    for mc in range(2):
                        for n in range(CHW // 512):
                            kps = psA.tile([P, 512], F32, tag="ps512")
                            for kc in range(2):
                                nc.tensor.matmul(
                                    kps[:],
                                    wk_b[:, kc, mc * P:(mc + 1) * P],
                                    xb[:, kc, n * 512:(n + 1) * 512],
                                    start=(kc == 0),
                                    stop=(kc == 1),
                                )
                            nc.scalar.activation(
                                kT_sb[ch][:, mc, n * 512:(n + 1) * 512], kps[:],
                                AF.Identity, bias=bk_sb[:, mc, :],
                            )
                    for mt in range(JPC):
                        vps = psA2.tile([P, C], F32, tag="ps256")
                        for kc in range(2):
                            nc.tensor.matmul(
                                vps[:],
                                xb[:, kc, mt * P:(mt + 1) * P],
                                wv_b[:, kc, :],
                                start=(kc == 0),
                                stop=(kc == 1),
                            )
                        nc.vector.tensor_tensor(
                            v_sb[ch][:, mt, :], vps[:], bvr_sb[:], ALU.add
                        )

            # ---- phase B: attention main loop ----
            with tc.tile_pool(name="otps", bufs=1, space="PSUM") as otps:
                ot = [otps.tile([P, R], F32, name=f"ot{i}") for i in range(2)]
                with (
                    tc.tile_pool(name="stps", bufs=2, space="PSUM") as stps,
                    tc.tile_pool(name="gfp", bufs=3) as gfp,
                    tc.tile_pool(name="gbp", bufs=3) as gbp,
                ):
                    gb_t = [None] * JT
                    SKEW = 2
                    for j in range(JT + SKEW):
                        if j < JT:
                            st = stps.tile([P, R], F32, tag="st")
                            for kc in range(2):
                                for rn in range(R // 512):
                                    nc.tensor.matmul(
                                        st[:, rn * 512:(rn + 1) * 512],
                                        kT_sb[j // JPC][:, kc, (j % JPC) * P:(j % JPC + 1) * P],
                                        qT_sb[:, kc, rn * 512:(rn + 1) * 512],
                                        start=(kc == 0),
                                        stop=(kc == 1),
                                    )
                            gf = gfp.tile([P, R], F32, tag="gf")
                            nc.scalar.activation(
                                gf[:], st[:], AF.Exp, scale=1.0 / np.float32(L),
                                accum_out=serow[:, j:j + 1],
                            )
                            gb = gbp.tile([P, R], BF16, tag="gb")
                            nc.vector.tensor_scalar(gb[:], gf[:], -1.0, None, ALU.add)
                            gb_t[j] = gb
                        if j >= SKEW:
                            jj = j - SKEW
                            for cc in range(2):
                                for rn in range(R // 512):
                                    nc.tensor.matmul(
                                        ot[cc][:, rn * 512:(rn + 1) * 512],
                                        v_sb[jj // JPC][:, jj % JPC, cc * P:(cc + 1) * P],
                                        gb_t[jj][:, rn * 512:(rn + 1) * 512],
                                        start=(jj == 0),
                                        stop=(jj == JT - 1),
                                    )

                # ---- phase C: epilogue ----
                with tc.tile_pool(name="psC", bufs=1, space="PSUM") as psC:
                    # exact f32 colsum(V) = Wv @ colsum(x) + L*bv
                    nc.vector.tensor_reduce(colx[:], xcs[:], AX.X, ALU.add)
                    for mc in range(2):
                        cvps = psC.tile([P, 1], F32, tag="cv")
                        for kc in range(2):
                            nc.tensor.matmul(
                                cvps[:],
                                wv_f[:, kc, mc * P:(mc + 1) * P],
                                colx[:, kc, :],
                                start=(kc == 0),
                                stop=(kc == 1),
                            )
                        nc.scalar.activation(
                            cv_sb[:, mc, :], cvps[:], AF.Identity,
                            bias=bvL_sb[:, mc, :],
                        )
                    # local sum of exp: reduce accum columns, then partitions
                    se1 = const.tile([P, 1], F32)
                    nc.vector.tensor_reduce(se1[:], serow[:], AX.X, ALU.add)
                    slps = psC.tile([1, 1], F32, tag="sl")
                    nc.tensor.matmul(slps[:], se1[:], ones_col[:])
                    nc.vector.tensor_copy(stats[0:1, 0:1], slps[:])
                    # global sum via AllReduce
                    ccin = dram.tile([1, 8], F32)
                    ccout = dram.tile([1, 8], F32)
                    nc.gpsimd.dma_start(ccin[:], stats[:])
                    nc.gpsimd.collective_compute(
                        "AllReduce",
                        ALU.add,
                        replica_groups=[list(range(NCORES))],
                        ins=[ccin.opt()],
                        outs=[ccout.opt()],
                    )
                    nc.gpsimd.dma_start(sg[:], ccout[:])
                    nc.vector.reciprocal(inv1[:], sg[0:1, 0:1])
                    bcps = psC.tile([P, 1], F32, tag="bc")
                    nc.tensor.matmul(bcps[:], ones_row[:], inv1[:])
                    nc.vector.tensor_copy(invb[:], bcps[:])
                    # out = (OT + colsumV) * (1/s)
                    for cc in range(2):
                        nc.vector.tensor_scalar(
                            out_sb[:, cc, :], ot[cc][:],
                            cv_sb[:, cc, :], invb[:],
                            ALU.add, ALU.mult,
                        )
                        nc.sync.dma_start(
                            out_d[cc * P:(cc + 1) * P, :], out_sb[:, cc, :]
                        )

    nc.compile()
    return nc


_CACHE = {}


def _get_nc():
    if "nc" not in _CACHE:
        _CACHE["nc"] = build()
    return _CACHE["nc"]


def _prep_in_maps(inputs):
    x = np.ascontiguousarray(np.asarray(inputs["x"], dtype=np.float32))
    Wq = np.asarray(inputs["Wq"], dtype=np.float32)
    bq = np.asarray(inputs["bq"], dtype=np.float32)
    Wk = np.asarray(inputs["Wk"], dtype=np.float32)
    bk = np.asarray(inputs["bk"], dtype=np.float32)
    Wv = np.asarray(inputs["Wv"], dtype=np.float32)
    bv = np.asarray(inputs["bv"], dtype=np.float32)

    xT = np.ascontiguousarray(x.T)
    common = {
        "xT": xT,
        "WqT": np.ascontiguousarray(Wq.T),
        "WkT": np.ascontiguousarray(Wk.T),
        "WvT": np.ascontiguousarray(Wv.T),
        "bq2": np.ascontiguousarray(bq[:, None]),
        "bk2": np.ascontiguousarray(bk[:, None]),
        "bvL": np.ascontiguousarray((np.float32(L) * bv)[:, None]),
        "bvrep": np.ascontiguousarray(np.tile(bv[None, :], (P, 1))),
    }
    in_maps = []
    for i in range(NCORES):
        m = dict(common)
        m["xTown"] = np.ascontiguousarray(xT[:, i * R:(i + 1) * R])
        in_maps.append(m)
    return in_maps


def _run(inputs, trace=False, **kw):
    nc = _get_nc()
    in_maps = _prep_in_maps(inputs)
    res = run_bass_kernel_spmd(nc, in_maps, list(range(NCORES)), trace=trace, **kw)
    parts = [np.asarray(res.results[i]["out"]).T for i in range(NCORES)]
    out = np.concatenate(parts, axis=0).astype(np.float32)
    return out, res


def kernel(**inputs):
    out, _ = _run(inputs, trace=False)
    return out


# revision 8
# speedup vs baseline: 1.2880x; 1.2880x over previous
"""Distributed Bass kernel for nn_Attention_94489280516 on 8 TRN2 NeuronCores.

Reference computation:
    q = x@Wq.T+bq; k = x@Wk.T+bk; v = x@Wv.T+bv          (x: [8192, 256])
    attn = softmax_global((q @ k.T) / 8192)               ([8192, 8192])
    out  = attn @ v                                       ([8192, 256])

Distribution: rows of q/out are sharded 1024/core. Each core projects its own
row block to k_local/v_local; one fp8 AllGather replicates full K^T and V to
every core. The global softmax needs one [128,4]-f32 AllReduce at the end
(sum of exp + global colsum(x) for the exact colsum(V) path).

Numerics: |a| < 0.03 structurally (a = q.k/8192, q,k ~ N(0,1)), so
    exp(a) = 1 + g,   g = exp(a)-1  computed in f32, scaled x8192 into fp8
    out_rows = (colsum(V) + G @ V) / sum_global(exp(a))
colsum(V) takes an exact f32 path (f32 colsum of x -> tiny f32 matmul with
Wv^T) because the output is dominated by that term. The big matmuls (QK^T and
G@V) run fp8 e4m3 with DoubleRow perf mode (K=256 per pass); projections are
bf16. Scales: q,k,v x16; g x8192; all folded into the final 1/s rescale.
"""

import os
import sys

for _p in ("/opt/trn_rl_repo", "/root/.axon_site/_ro/trn_rl_repo"):
    if os.path.isdir(_p) and _p not in sys.path:
        sys.path.insert(0, _p)

import numpy as np

import concourse.bass as bass
import concourse.bacc as bacc
import concourse.mybir as mybir
import concourse.tile as tile
from concourse.bass_utils import run_bass_kernel_spmd

F32 = mybir.dt.float32
BF16 = mybir.dt.bfloat16
FP8 = mybir.dt.float8e4
AF = mybir.ActivationFunctionType
ALU = mybir.AluOpType
AX = mybir.AxisListType
DR = mybir.MatmulPerfMode.DoubleRow

L = 8192          # total rows
C = 256           # channels
NCORES = 8
R = L // NCORES   # 1024 rows per core
P = 128
JT = L // P       # 64 key tiles
NPAIR = JT // 2   # 32 key tile pairs (fp8 DoubleRow contracts 256 keys)
TPB = R // P      # 8 key tiles per gathered block

SQ = 16.0         # q scale into fp8
SK = 16.0         # k scale
SV = 16.0         # v scale
SG = 8192.0       # g scale
SGSV = SG * SV
EXPSCALE = 1.0 / (L * SQ * SK)
NTOT = float(L) * float(L)


def build():
    nc = bacc.Bacc(None, num_devices=NCORES)

    xo_d = nc.declare_dram_parameter("xTown", [C, R], F32, isOutput=False)
    wq_d = nc.declare_dram_parameter("WqT", [C, C], F32, isOutput=False)
    wk_d = nc.declare_dram_parameter("WkT", [C, C], F32, isOutput=False)
    wv_d = nc.declare_dram_parameter("WvT", [C, C], F32, isOutput=False)
    bq_d = nc.declare_dram_parameter("bq2", [C, 1], F32, isOutput=False)
    bk_d = nc.declare_dram_parameter("bk2", [C, 1], F32, isOutput=False)
    bvL_d = nc.declare_dram_parameter("bvL", [C, 1], F32, isOutput=False)
    bvr_d = nc.declare_dram_parameter("bvr16", [P, C], F32, isOutput=False)
    out_d = nc.declare_dram_parameter("out", [C, R], F32, isOutput=True)

    with tile.TileContext(nc) as tc:
        with (
            tc.tile_pool(name="const", bufs=1) as const,
            tc.tile_pool(name="big", bufs=1) as big,
            tc.tile_pool(name="dram", bufs=1, space="DRAM") as dram,
        ):
            # ---- persistent tiles ----
            wq_f = const.tile([P, 2, C], F32)
            wk_f = const.tile([P, 2, C], F32)
            wv_f = const.tile([P, 2, C], F32)
            wq_b = const.tile([P, 2, C], BF16)
            wk_b = const.tile([P, 2, C], BF16)
            wv_b = const.tile([P, 2, C], BF16)
            bq_sb = const.tile([P, 2, 1], F32)
            bk_sb = const.tile([P, 2, 1], F32)
            bvL_sb = const.tile([P, 2, 1], F32)
            bvr_sb = const.tile([P, C], F32)
            ones_col = const.tile([P, 1], F32)
            ones_row = const.tile([1, P], F32)
            serow = const.tile([P, JT], F32)
            xcs = const.tile([P, 2, 1], F32)
            stats4 = const.tile([P, 4], F32)
            sgl4 = const.tile([P, 4], F32)
            cv_sb = const.tile([P, 2, 1], F32)
            sval = const.tile([1, 1], F32)
            inv1 = const.tile([1, 1], F32)
            invb = const.tile([P, 1], F32)
            out_sb = const.tile([P, 2, R], F32)
            qT_sb = big.tile([P, 2, R], FP8)
            kTl_sb = big.tile([P, 2, R], FP8)
            vl_sb = big.tile([P, TPB, C], FP8)
            kT_sb = big.tile([P, 2, NCORES, R], FP8)
            v_sb = big.tile([P, NCORES, TPB, C], FP8)

            agin = dram.tile([2, C, R], FP8)
            agout = dram.tile([NCORES, 2, C, R], FP8, addr_space="Shared")
            ccin = dram.tile([P, 4], F32)
            ccout = dram.tile([P, 4], F32)

            for kc in range(2):
                nc.sync.dma_start(wq_f[:, kc, :], wq_d[kc * P:(kc + 1) * P, :])
                nc.sync.dma_start(wk_f[:, kc, :], wk_d[kc * P:(kc + 1) * P, :])
                nc.sync.dma_start(wv_f[:, kc, :], wv_d[kc * P:(kc + 1) * P, :])
                nc.sync.dma_start(bq_sb[:, kc, :], bq_d[kc * P:(kc + 1) * P, :])
                nc.sync.dma_start(bk_sb[:, kc, :], bk_d[kc * P:(kc + 1) * P, :])
                nc.sync.dma_start(bvL_sb[:, kc, :], bvL_d[kc * P:(kc + 1) * P, :])
            nc.sync.dma_start(bvr_sb[:], bvr_d[:, :])
            nc.vector.tensor_copy(wq_b[:], wq_f[:])
            nc.vector.tensor_copy(wk_b[:], wk_f[:])
            nc.vector.tensor_copy(wv_b[:], wv_f[:])
            nc.vector.memset(ones_col[:], 1.0)
            nc.vector.memset(ones_row[:], 1.0 / SGSV)
            nc.vector.memset(stats4[:], 0.0)

            # ---- phase A: local projections (bf16 matmuls -> fp8 tiles) ----
            with (
                tc.tile_pool(name="xload", bufs=1) as xload,
                tc.tile_pool(name="psA", bufs=4, space="PSUM") as psA,
                tc.tile_pool(name="psA2", bufs=2, space="PSUM") as psA2,
            ):
                xo_f = xload.tile([P, 2, R], F32)
                for kc in range(2):
                    nc.sync.dma_start(xo_f[:, kc, :], xo_d[kc * P:(kc + 1) * P, :])
                xo_b = xload.tile([P, 2, R], BF16)
                nc.vector.tensor_copy(xo_b[:], xo_f[:])
                nc.vector.tensor_reduce(xcs[:, :, 0], xo_f[:], AX.X, ALU.add)

                # k^T_local and V_local first (feed the AllGather)
                for mc in range(2):
                    for rn in range(R // 512):
                        kps = psA.tile([P, 512], F32, tag="ps512")
                        for kc in range(2):
                            nc.tensor.matmul(
                                kps[:],
                                wk_b[:, kc, mc * P:(mc + 1) * P],
                                xo_b[:, kc, rn * 512:(rn + 1) * 512],
                                start=(kc == 0),
                                stop=(kc == 1),
                            )
                        nc.vector.tensor_scalar(
                            kTl_sb[:, mc, rn * 512:(rn + 1) * 512], kps[:],
                            bk_sb[:, mc, :], SK, ALU.add, ALU.mult,
                        )
                for mt in range(TPB):
                    vps = psA2.tile([P, C], F32, tag="ps256")
                    for kc in range(2):
                        nc.tensor.matmul(
                            vps[:],
                            xo_b[:, kc, mt * P:(mt + 1) * P],
                            wv_b[:, kc, :],
                            start=(kc == 0),
                            stop=(kc == 1),
                        )
                    nc.vector.scalar_tensor_tensor(
                        vl_sb[:, mt, :], vps[:], SV, bvr_sb[:], ALU.mult, ALU.add
                    )

                # ship local k/v to the AllGather bounce
                for kc in range(2):
                    nc.sync.dma_start(
                        agin[0, kc * P:(kc + 1) * P, :], kTl_sb[:, kc, :]
                    )
                agin_v = agin[1].rearrange("a b -> (a b)").rearrange(
                    "(t p c) -> p t c", t=TPB, p=P, c=C
                )
                nc.sync.dma_start(agin_v, vl_sb[:])
                nc.gpsimd.collective_compute(
                    "AllGather",
                    ALU.bypass,
                    replica_groups=[list(range(NCORES))],
                    ins=[agin.opt()],
                    outs=[agout.opt()],
                )
                # gathered K^T: [b][kc*128+k, col] -> kT_sb[k, kc, b, col]
                for b in range(NCORES):
                    agout_k = agout[b, 0, :, :].rearrange(
                        "(kc k) col -> k kc col", kc=2, k=P
                    )
                    nc.sync.dma_start(kT_sb[:, :, b, :], agout_k)
                    # gathered V block: flat (t p c) -> v_sb[p, b, t, c]
                    agout_v = agout[b, 1, :, :].rearrange("a c -> (a c)").rearrange(
                        "(t p c) -> p t c", t=TPB, p=P, c=C
                    )
                    nc.sync.dma_start(v_sb[:, b, :, :], agout_v)

                # q projection (own rows only)
                for mc in range(2):
                    for rn in range(R // 512):
                        qps = psA.tile([P, 512], F32, tag="ps512")
                        for kc in range(2):
                            nc.tensor.matmul(
                                qps[:],
                                wq_b[:, kc, mc * P:(mc + 1) * P],
                                xo_b[:, kc, rn * 512:(rn + 1) * 512],
                                start=(kc == 0),
                                stop=(kc == 1),
                            )
                        nc.vector.tensor_scalar(
                            qT_sb[:, mc, rn * 512:(rn + 1) * 512], qps[:],
                            bq_sb[:, mc, :], SQ, ALU.add, ALU.mult,
                        )

            # ---- phase B: attention main loop (fp8 DoubleRow) ----
            with tc.tile_pool(name="otps", bufs=1, space="PSUM") as otps:
                ot = [otps.tile([P, R], F32, name=f"ot{i}") for i in range(2)]

                def av_pair(p):
                    b, t0 = p // (TPB // 2), (p % (TPB // 2)) * 2
                    for cc in range(2):
                        for rn in range(R // 512):
                            nc.tensor.matmul(
                                ot[cc][:, rn * 512:(rn + 1) * 512],
                                v_sb[:, b, t0:t0 + 2, cc * P:(cc + 1) * P],
                                gb_t[p][:, :, rn * 512:(rn + 1) * 512],
                                start=(p == 0),
                                stop=(p == NPAIR - 1),
                                perf_mode=DR,
                            )

                with (
                    tc.tile_pool(name="stps", bufs=2, space="PSUM") as stps,
                    tc.tile_pool(name="gfp", bufs=3) as gfp,
                    tc.tile_pool(name="gbp", bufs=3) as gbp,
                ):
                    gb_t = [None] * NPAIR
                    for j in range(JT):
                        st = stps.tile([P, R], F32, tag="st")
                        for rn in range(R // 512):
                            nc.tensor.matmul(
                                st[:, rn * 512:(rn + 1) * 512],
                                kT_sb[:, :, j // TPB, (j % TPB) * P:(j % TPB + 1) * P],
                                qT_sb[:, :, rn * 512:(rn + 1) * 512],
                                start=True,
                                stop=True,
                                perf_mode=DR,
                            )
                        gf = gfp.tile([P, R], F32, tag="gf")
                        nc.scalar.activation(
                            gf[:], st[:], AF.Exp, scale=EXPSCALE,
                            accum_out=serow[:, j:j + 1],
                        )
                        if j % 2 == 0:
                            gb2 = gbp.tile([P, 2, R], FP8, tag="gb")
                            gb_t[j // 2] = gb2
                        nc.vector.tensor_scalar(
                            gb_t[j // 2][:, j % 2, :], gf[:], -1.0, SG,
                            ALU.add, ALU.mult,
                        )
                        if j >= 3 and j % 2 == 1:
                            av_pair((j - 3) // 2)
                    av_pair(NPAIR - 2)
                    av_pair(NPAIR - 1)

                # ---- phase C: epilogue ----
                with tc.tile_pool(name="psC", bufs=1, space="PSUM") as psC:
                    nc.vector.tensor_copy(stats4[:, 0:2], xcs[:, :, 0])
                    nc.vector.tensor_reduce(
                        stats4[:, 2:3], serow[:], AX.X, ALU.add
                    )
                    nc.gpsimd.dma_start(ccin[:], stats4[:])
                    nc.gpsimd.collective_compute(
                        "AllReduce",
                        ALU.add,
                        replica_groups=[list(range(NCORES))],
                        ins=[ccin.opt()],
                        outs=[ccout.opt()],
                    )
                    nc.gpsimd.dma_start(sgl4[:], ccout[:])
                    # colsum(V)*SGSV = (Wv @ colsum_x + L*bv) * SGSV
                    for mc in range(2):
                        cvps = psC.tile([P, 1], F32, tag="cv")
                        for kc in range(2):
                            nc.tensor.matmul(
                                cvps[:],
                                wv_f[:, kc, mc * P:(mc + 1) * P],
                                sgl4[:, kc:kc + 1],
                                start=(kc == 0),
                                stop=(kc == 1),
                            )
                        nc.vector.tensor_scalar(
                            cv_sb[:, mc, :], cvps[:],
                            bvL_sb[:, mc, :], SGSV, ALU.add, ALU.mult,
                        )
                    # s = sum(exp): sum over partitions of gathered row sums
                    slps = psC.tile([1, 1], F32, tag="sl")
                    nc.tensor.matmul(slps[:], sgl4[:, 2:3], ones_col[:])
                    nc.vector.tensor_copy(sval[:], slps[:])
                    nc.vector.reciprocal(inv1[:], sval[:])
                    # broadcast 1/(s*SGSV) to all partitions via ones matmul
                    bcps = psC.tile([P, 1], F32, tag="bc")
                    nc.tensor.matmul(bcps[:], ones_row[:], inv1[:])
                    nc.vector.tensor_copy(invb[:], bcps[:])
                    # out = (OT + colsumV*SGSV) / (s*SGSV)
                    for cc in range(2):
                        nc.vector.tensor_scalar(
                            out_sb[:, cc, :], ot[cc][:],
                            cv_sb[:, cc, :], invb[:],
                            ALU.add, ALU.mult,
                        )
                        nc.sync.dma_start(
                            out_d[cc * P:(cc + 1) * P, :], out_sb[:, cc, :]
                        )

    nc.compile()
    return nc


_CACHE = {}


def _get_nc():
    if "nc" not in _CACHE:
        _CACHE["nc"] = build()
    return _CACHE["nc"]


def _prep_in_maps(inputs):
    x = np.ascontiguousarray(np.asarray(inputs["x"], dtype=np.float32))
    Wq = np.asarray(inputs["Wq"], dtype=np.float32)
    bq = np.asarray(inputs["bq"], dtype=np.float32)
    Wk = np.asarray(inputs["Wk"], dtype=np.float32)
    bk = np.asarray(inputs["bk"], dtype=np.float32)
    Wv = np.asarray(inputs["Wv"], dtype=np.float32)
    bv = np.asarray(inputs["bv"], dtype=np.float32)

    xT = np.ascontiguousarray(x.T)
    common = {
        "WqT": np.ascontiguousarray(Wq.T),
        "WkT": np.ascontiguousarray(Wk.T),
        "WvT": np.ascontiguousarray(Wv.T),
        "bq2": np.ascontiguousarray(bq[:, None]),
        "bk2": np.ascontiguousarray(bk[:, None]),
        "bvL": np.ascontiguousarray((np.float32(L) * bv)[:, None]),
        "bvr16": np.ascontiguousarray(np.float32(SV) * np.tile(bv[None, :], (P, 1))),
    }
    in_maps = []
    for i in range(NCORES):
        m = dict(common)
        m["xTown"] = np.ascontiguousarray(xT[:, i * R:(i + 1) * R])
        in_maps.append(m)
    return in_maps


def _run(inputs, trace=False, **kw):
    nc = _get_nc()
    in_maps = _prep_in_maps(inputs)
    res = run_bass_kernel_spmd(nc, in_maps, list(range(NCORES)), trace=trace, **kw)
    parts = [np.asarray(res.results[i]["out"]).T for i in range(NCORES)]
    out = np.concatenate(parts, axis=0).astype(np.float32)
    return out, res


def kernel(**inputs):
    out, _ = _run(inputs, trace=False)
    return out
